# revision 12
# baseline (speedup 1.0000x reference)
"""Trainium2 Bass kernel for causal multi-head attention (prefill).

Problem: x[2,2048,768], 12 heads x 64 dim, causal softmax(QK^T/8)V + out-proj.

Sharding (8 cores, no collectives): core c handles batch c//4 and head group
c%4 (3 heads).  Each core computes, for its batch b and heads hs:
    qT,kT = (Wq_hs @ x_b^T), (Wk_hs @ x_b^T)        [192, 2048] (transposed)
    v     = x_b @ Wv_hs^T                            [2048, 192+ones]
    expT  = exp(scoresT/8) masked causally           [kv, sq] per head
    ctxT_h = v_aug^T @ expT  (extra row = softmax denom via ones column)
    outT_partial = Wo[:,cols_hs] @ (ctxT/den)        [768, 2048]
Host sums the 4 partial outputs per batch and transposes back.

v4, built on hw microbenchmarks of the PE:
  - K=64 matmuls on alternating row-groups run 2x (concurrent tiles);
    weight reloads are free only for 128-column weights (FWL+background
    buffer), and a 65-column weight pays a ~120ns serial load.
  - So: head2's q/k live in BOTH row-group halves (dual copies via a
    partition-shift DMA) and the per-kv-tile score matmuls alternate
    groups perfectly: (g0,g64,g0) / (g64,g0,g64) by kv-tile parity.
  - ctx weights padded to 128 columns (vaug head stride 128, zero pad);
    output rows 65..127 of the ctx psum are dead.
  - Emission is software-pipelined (scores(i), exp(i), ctx(i-1)) so the
    Tile scheduler keeps score triplets adjacent (ctx emitted earlier
    would slot between them, breaking tile concurrency).
  - bf16 datapath; PSUM + softmax normalization fp32.
"""

import numpy as np

import concourse.bass as bass
import concourse.tile as tile
from concourse import bacc, mybir
from concourse.bass_utils import run_bass_kernel_spmd

F32 = mybir.dt.float32
BF16 = mybir.dt.bfloat16

B, S, D = 2, 2048, 768
H, DH = 12, 64
HPC = 3                 # heads per core
GH = HPC * DH           # 192 head dims per core
NCORES = 8
KT = D // 128           # 6 contraction tiles for projections
WJ = 512                # q window width
NJ = S // WJ            # 4 windows
NKV = S // 128          # 16 kv tiles of 128


def build():
    nc = bacc.Bacc("TRN2", target_bir_lowering=False, debug=False)

    # host pre-packed input: per partition line p:
    #   [ x (4 nt-blocks x 6 k x 512) | wq 6x128 | wk 6x128 | wqk2 6x128
    #     | wv 6x256 ]  (wqk2 cols 0:64 = Wk2^T, 64:128 = Wq2^T)
    # few BIG transfers: each dma_start costs ~0.9us of serial sync-engine
    # issue, so input lands in ~6 DMAs, nt0 columns first.
    XW = 4 * KT * 512 + 3 * KT * 128 + KT * 256
    xw = nc.dram_tensor("xw", [128, XW], BF16, kind="ExternalInput")
    wo = nc.dram_tensor("wo", [GH, D], BF16, kind="ExternalInput")
    tri = nc.dram_tensor("tri", [128, 128], BF16, kind="ExternalInput")
    onesd = nc.dram_tensor("onesd", [1, 64], BF16, kind="ExternalInput")
    outT = nc.dram_tensor("outT", [D, S], F32, kind="ExternalOutput")

    with tile.TileContext(nc) as tc, \
         nc.allow_low_precision(reason="bf16 datapath, fp32 psum/normalize"):
        with tc.tile_pool(name="sb", bufs=1) as sb, \
             tc.tile_pool(name="sbe", bufs=6) as sbe, \
             tc.tile_pool(name="sbo", bufs=3) as sbo, \
             tc.tile_pool(name="sbn", bufs=2) as sbn, \
             tc.tile_pool(name="ps", bufs=2, space="PSUM") as ps, \
             tc.tile_pool(name="psp", bufs=3, space="PSUM") as psp, \
             tc.tile_pool(name="psc", bufs=3, space="PSUM") as psc:

            # ---- phase 0: load weights + x ----
            # xsb is [128, nt, k, 512]: all k-tiles of one 512-col window
            # arrive in one DMA so window 0 compute starts ~10us in.
            xsb = sb.tile([128, 4, KT, 512], BF16, tag="xsb")
            wqkx_sb = sb.tile([128, 3, KT, 128], BF16, tag="wqkx")
            wv_sb = sb.tile([128, KT, 256], BF16, tag="wv")
            ones64 = sb.tile([65, 64], BF16, tag="ones64")
            nc.sync.dma_start(ones64[64:65, :], onesd[:, :])
            XOFF = 4 * KT * 512
            nc.sync.dma_start(
                wqkx_sb,
                xw[:, XOFF:XOFF + 3 * KT * 128].rearrange(
                    "p (w k m) -> p w k m", w=3, k=KT))
            xw_x = xw[:, 0:XOFF].rearrange("p (t k n) -> p t k n", t=4, k=KT)
            nc.sync.dma_start(xsb[:, 0], xw_x[:, 0])
            nc.sync.dma_start(
                wv_sb,
                xw[:, XOFF + 3 * KT * 128:XW].rearrange(
                    "p (k m) -> p k m", k=KT))
            tri_sb = sb.tile([128, 128], BF16, tag="tri")
            nc.sync.dma_start(tri_sb, tri[:, :])
            for nt in range(1, 4):
                nc.sync.dma_start(xsb[:, nt], xw_x[:, nt])
            wo01_sb = sb.tile([128, D], BF16, tag="wo01")
            wo2_sb = sb.tile([64, D], BF16, tag="wo2")
            nc.sync.dma_start(wo01_sb, wo[0:128, :])
            nc.sync.dma_start(wo2_sb, wo[128:GH, :])
            wq_sb, wk_sb, wqk2_sb = (wqkx_sb[:, 0], wqkx_sb[:, 1],
                                     wqkx_sb[:, 2])
            # preload the exp ACT table set (~2.7us) while DMA streams
            warm = sbn.tile([65, 64], F32, tag="warm", name="warm")
            nc.scalar.activation(warm[64:65, 0:2], ones64[64:65, 0:2],
                                 mybir.ActivationFunctionType.Exp, scale=1.0)

            # ---- phase 1: projections ----
            # heads 0/1: qt/kt [128, S] (h0 rows 0:64, h1 rows 64:128).
            # head 2: dual-group tiles q2d/k2d [128, S] - the same 64 rows
            # replicated in both halves so scores can alternate row-groups.
            qt_sb = sb.tile([128, S], BF16, tag="qt")
            kt_sb = sb.tile([128, S], BF16, tag="kt")
            q2d = sb.tile([128, S], BF16, tag="q2d")
            k2d = sb.tile([128, S], BF16, tag="k2d")

            def proj_qk(nt, kouter=False):
                ntw = slice(nt * 512, (nt + 1) * 512)
                trips = ((qt_sb, wq_sb), (kt_sb, wk_sb), (None, wqk2_sb))
                if kouter:
                    # first window: 3 psum chains accumulate per arriving
                    # x k-tile so the PE starts ~2us into the input DMA
                    pool = psp
                    pps = [pool.tile([128, 512], F32, tag="sp", name="pp")
                           for _ in trips]
                    for k in range(KT):
                        for (dst, wsb), pp in zip(trips, pps):
                            nc.tensor.matmul(pp, wsb[:, k, :], xsb[:, nt, k, :],
                                             start=(k == 0), stop=(k == KT - 1))
                else:
                    pps = []
                    for dst, wsb in trips:
                        pp = ps.tile([128, 512], F32, tag="sc", name="pp")
                        for k in range(KT):
                            nc.tensor.matmul(pp, wsb[:, k, :], xsb[:, nt, k, :],
                                             start=(k == 0), stop=(k == KT - 1))
                        pps.append(pp)
                nc.vector.tensor_copy(qt_sb[:, ntw], pps[0])
                nc.vector.tensor_copy(kt_sb[:, ntw], pps[1])
                # packed mt1: psum rows 0:64 = k2, rows 64:128 = q2
                nc.vector.tensor_copy(k2d[0:64, ntw], pps[2][0:64, :])
                nc.vector.tensor_copy(q2d[64:128, ntw], pps[2][64:128, :])
                # replicate into the other row-group half (partition shift)
                nc.sync.dma_start(k2d[64:128, ntw], k2d[0:64, ntw])
                nc.sync.dma_start(q2d[0:64, ntw], q2d[64:128, ntw])

            # v_aug: [128, NKV, 384]; head h: v at cols 128h..128h+63, ones
            # at col 128h+64, zeros 128h+65..128h+127 (128-col weight => FWL)
            vaug = sb.tile([128, NKV, 384], BF16, tag="vaug")
            for h in range(HPC):
                nc.vector.memset(vaug[:, :, 128 * h + 65:128 * (h + 1)], 0.0)
                nc.scalar.activation(
                    vaug[:, :, 128 * h + 64:128 * h + 65],
                    tri_sb[:, h * NKV:(h + 1) * NKV].rearrange(
                        "p (t c) -> p t c", c=1),
                    mybir.ActivationFunctionType.Copy, bias=1.0, scale=0.0)

            def proj_v(i):
                pp = ps.tile([128, 256], F32, tag="sc", name="pp")
                c0 = (i % 4) * 128
                for k in range(KT):
                    nc.tensor.matmul(
                        pp,
                        xsb[:, i // 4, k, c0:c0 + 128],
                        wv_sb[:, k, :],
                        start=(k == 0), stop=(k == KT - 1))
                nc.vector.tensor_copy(
                    vaug[:, i, :].rearrange("p (h c) -> p h c", c=128)[:, :, 0:64],
                    pp[:, 0:192].rearrange("p (h c) -> p h c", c=64))

            # ---- phase 2/3 helpers ----
            # row-group per (head, kv-tile parity): h0 -> g0, h1 -> g64,
            # h2 -> g(i%2).  Emission order alternates groups exactly.
            def kslc(h, i):
                if h == 0:
                    return kt_sb[0:64, i * 128:(i + 1) * 128]
                if h == 1:
                    return kt_sb[64:128, i * 128:(i + 1) * 128]
                g = 64 * (i % 2)
                return k2d[g:g + 64, i * 128:(i + 1) * 128]

            def qslc(h, i, c0, c1):
                if h == 0:
                    return qt_sb[0:64, c0:c1]
                if h == 1:
                    return qt_sb[64:128, c0:c1]
                g = 64 * (i % 2)
                return q2d[g:g + 64, c0:c1]

            ctxT01 = sb.tile([128, S], BF16, tag="ctxT01")
            ctxT2 = sb.tile([64, S], BF16, tag="ctxT2")

            def chains(J):
                """scores+exp+mask for kv-tile i, ctx for i-1 (software
                pipeline keeps the score triplet adjacent on the PE queue)."""
                ctxp = [psc.tile([128, WJ], F32, tag="ctx", name=f"ctx{J}_{h}")
                        for h in range(HPC)]
                imax = 4 * J + 3

                def emit_ctx(i, i_esbs):
                    for h in range(HPC):
                        nc.tensor.matmul(
                            ctxp[h][:, max(0, 128 * i - WJ * J):WJ],
                            vaug[:, i, 128 * h:128 * h + 128],
                            i_esbs[h],
                            start=(i == 0), stop=(i == imax))

                prev = None
                for i in range(imax + 1):
                    d = 128 * i - WJ * J
                    col0 = max(0, d)
                    order = (0, 1, 2) if i % 2 == 0 else (1, 0, 2)
                    sps = {}
                    for h in order:
                        spsum = psp.tile([128, WJ], F32, tag="sp", name="sp")
                        nc.tensor.matmul(
                            spsum[:, col0:WJ],
                            kslc(h, i),
                            qslc(h, i, WJ * J + col0, WJ * (J + 1)),
                            start=True, stop=True)
                        sps[h] = spsum
                    esbs = []
                    for h in range(HPC):
                        esb = sbe.tile([128, WJ], BF16, tag="exp", name="esb")
                        nc.scalar.activation(
                            esb[:, col0:WJ], sps[h][:, col0:WJ],
                            mybir.ActivationFunctionType.Exp, scale=0.125)
                        if d >= 0:
                            nc.vector.tensor_mul(
                                esb[:, d:d + 128], esb[:, d:d + 128], tri_sb)
                        esbs.append(esb[:, col0:WJ])
                    if prev is not None:
                        emit_ctx(i - 1, prev)
                    prev = esbs
                emit_ctx(imax, prev)
                return ctxp

            def norms(J, ctxp):
                """normalize rows 0:64 by row 64 (softmax denominator).
                PE only runs the K=1 den-broadcast; recip + final mul on DVE.
                (walrus: a tensor op may read at most one PSUM operand, so
                the reciprocal lands in SBUF before the multiply.)"""
                h1t = None
                for h in range(HPC):
                    denr = sbn.tile([65, WJ], BF16, tag="denr", name="denr")
                    nc.vector.tensor_copy(denr[64:65, :], ctxp[h][64:65, :])
                    bps = ps.tile([64, WJ], F32, tag="sc", name="bps")
                    nc.tensor.matmul(bps, ones64[64:65, :], denr[64:65, :],
                                     start=True, stop=True)
                    invb = sbn.tile([64, WJ], F32, tag="invb", name="invb")
                    nc.vector.reciprocal_approx_fast(invb, bps)
                    if h == 0:
                        dst = ctxT01[0:64, WJ * J:WJ * (J + 1)]
                    elif h == 1:
                        h1t = sbo.tile([64, WJ], BF16, tag="h1t", name="h1t")
                        dst = h1t
                    else:
                        dst = ctxT2[:, WJ * J:WJ * (J + 1)]
                    nc.vector.tensor_mul(dst, ctxp[h][0:64, :], invb)
                # partition-shift h1's ctxT into rows 64:128
                nc.sync.dma_start(ctxT01[64:128, WJ * J:WJ * (J + 1)], h1t)

            def outproj(J):
                for mt in range(6):
                    ops = ps.tile([128, WJ], F32, tag="sc", name="ops")
                    nc.tensor.matmul(
                        ops, wo01_sb[:, mt * 128:(mt + 1) * 128],
                        ctxT01[:, J * WJ:(J + 1) * WJ],
                        start=True, stop=False)
                    nc.tensor.matmul(
                        ops, wo2_sb[:, mt * 128:(mt + 1) * 128],
                        ctxT2[:, J * WJ:(J + 1) * WJ],
                        start=False, stop=True)
                    osb = sbo.tile([128, WJ], F32, tag="osb", name="osb")
                    nc.vector.tensor_copy(osb, ops)
                    nc.sync.dma_start(
                        outT[mt * 128:(mt + 1) * 128, J * WJ:(J + 1) * WJ],
                        osb)

            # ---- interleaved schedule ----
            # proj nt / kv-tiles feed window J as soon as available; norms(J)
            # ride behind chains(J); outproj(J) behind chains(J+1).
            proj_qk(0, kouter=True)
            for i in range(4):
                proj_v(i)
            ctx0 = chains(0)
            proj_qk(1)
            for i in range(4, 8):
                proj_v(i)
            norms(0, ctx0)
            ctx1 = chains(1)
            proj_qk(2)
            for i in range(8, 12):
                proj_v(i)
            norms(1, ctx1)
            outproj(0)
            ctx2 = chains(2)
            proj_qk(3)
            for i in range(12, 16):
                proj_v(i)
            norms(2, ctx2)
            outproj(1)
            ctx3 = chains(3)
            norms(3, ctx3)
            outproj(2)
            outproj(3)

    nc.compile()
    return nc


def shard_inputs(x, Wq, Wk, Wv, Wo):
    import ml_dtypes
    bf16 = ml_dtypes.bfloat16

    def krearrange(wT, cols):
        # [D, cols] -> [128, KT*cols]; line p holds wT[k*128+p, :] for all k
        return np.ascontiguousarray(
            wT.reshape(KT, 128, cols).transpose(1, 0, 2).reshape(128, KT * cols)
        ).astype(bf16)

    x = np.asarray(x, np.float32)
    tri = np.triu(np.ones((128, 128), np.float32)).astype(bf16)
    ones = np.ones((1, 64), np.float32).astype(bf16)
    in_maps = []
    for c in range(NCORES):
        b, g = c // 4, c % 4
        rs = slice(GH * g, GH * g + GH)
        wqT = np.ascontiguousarray(np.asarray(Wq, np.float32)[rs].T)  # [D, 192]
        wkT = np.ascontiguousarray(np.asarray(Wk, np.float32)[rs].T)
        wqk2 = np.concatenate([wkT[:, 128:192], wqT[:, 128:192]], axis=1)
        wv_t = np.concatenate(
            [np.ascontiguousarray(np.asarray(Wv, np.float32)[rs].T),
             np.zeros((D, 64), np.float32)], axis=1)
        xT = np.ascontiguousarray(x[b].T)                     # [D, S]
        xb = xT.reshape(KT, 128, S)
        xparts = [np.ascontiguousarray(
                      xb[:, :, nt * 512:(nt + 1) * 512]
                  ).transpose(1, 0, 2).reshape(128, KT * 512)
                  for nt in range(4)]
        xw_host = np.concatenate(
            xparts + [krearrange(np.ascontiguousarray(wqT[:, 0:128]), 128),
                      krearrange(np.ascontiguousarray(wkT[:, 0:128]), 128),
                      krearrange(np.ascontiguousarray(wqk2), 128),
                      krearrange(wv_t, 256)], axis=1)
        in_maps.append({
            "xw": np.ascontiguousarray(xw_host).astype(bf16),
            "wo": np.ascontiguousarray(np.asarray(Wo, np.float32)[:, rs].T).astype(bf16),
            "tri": tri,
            "onesd": ones,
        })
    return in_maps


def assemble(results, bo):
    out = np.zeros((B, S, D), np.float32)
    for c in range(NCORES):
        out[c // 4] += results[c]["outT"].T
    return out + np.asarray(bo, np.float32)[None, None, :]


_NC = None


def kernel(x, Wq, Wk, Wv, Wo, bo, **run_kwargs):
    global _NC
    if _NC is None:
        _NC = build()
    in_maps = shard_inputs(x, Wq, Wk, Wv, Wo)
    res = run_bass_kernel_spmd(_NC, in_maps, core_ids=list(range(NCORES)),
                               **run_kwargs)
    out = assemble(res.results, bo)
    kernel.last_results = res
    return out
